# revision 9
# baseline (speedup 1.0000x reference)
"""AdaptiveVectorQuantizer Trainium2 kernel (8 NeuronCores, data-parallel).

v2: PE-lean rewrite of the baseline.
 - Scoring: fp16 hi/lo split of x and -2*cb.T -> 3 bf16-rate matmuls
   (1 cyc/col) instead of fp32 LOW_HIGH (4 cyc/col).  score = e_sq - 2 x.e
   exact to ~1e-8 (fp16 products are exact in fp32 PSUM accumulate).
 - Argmin: per-block min-reduce -> prefix-min via tensor_tensor_scan
   (reset-slot trick), eq-mask (fp32, split DVE/GPSIMD), mask * (code+1)
   in bf16 (exact ints <= 256), max-reduce extract (first-tie goes to the
   larger index - rare, within tolerance), select-chain for per-level idx.
 - Selector/gather matmuls all bf16 (operands are exact small ints or
   codebook values that round to bf16 output anyway).  The Relu bias
   (code^2-1 split hi/lo) rides in the selector matmul via two ones-rows,
   so one activation (scale=-1, bias=0) thresholds a whole [128,2048] gq.
 - Gather psum drained by per-span copies split ACT/DVE; output staged
   bf16, DMA'd per half-image.

Combined-onehot tiles (rows must be 32-aligned for engine slicing):
  T1: rows 0-31 l4 | 32-95 l5 | 96-125 l0..l3 | 126-127 pad
  T2: l6 codes 0-127.  T3: l7 codes 0-127.  T4: l7 codes 128-255.
"""

import sys
import numpy as np

sys.path.insert(0, "/opt/trn_rl_repo")

B, C, H, W = 32, 64, 64, 64
P, D = 256, 64
NCORES = 8
IMGS = B // NCORES            # 4 images per core
HWTOK = H * W                 # 4096 tokens per image
NTILE = HWTOK // 128          # 32
NSPAN = HWTOK // 512          # 8
NLVL = 8
BIGF = 1.0e30
MAGIC = float(2 ** 23)

# T1 row layout: rows 0-31: l4; 32-95: l5; 96-125: l0..l3; 126-127 pad
_T1_ROWS = [(4, j) for j in range(32)]
_T1_ROWS += [(5, j) for j in range(64)]
for lvl, k in ((0, 2), (1, 4), (2, 8), (3, 16)):
    _T1_ROWS += [(lvl, j) for j in range(k)]
_T1_ROWS += [(-1, 0), (-1, 0)]                     # rows 126-127 pad
assert len(_T1_ROWS) == 128


def _tile_maps():
    t1l = np.array([r[0] for r in _T1_ROWS])
    t1c = np.array([r[1] for r in _T1_ROWS])
    return [
        (t1l, t1c),
        (np.full(128, 6), np.arange(128)),            # T2: level 6
        (np.full(128, 7), np.arange(128)),            # T3: level 7 lo
        (np.full(128, 7), np.arange(128) + 128),      # T4: level 7 hi
    ]


def _bf16(a):
    import ml_dtypes
    return np.asarray(a, dtype=np.float32).astype(ml_dtypes.bfloat16)


def _host_consts(codebook):
    cb = np.ascontiguousarray(codebook, dtype=np.float32)         # [256, 64]
    esq = np.sum(cb.astype(np.float64) * cb, axis=1)              # [256]
    m2c = (-2.0 * cb.T).astype(np.float32)                        # [64, 256]

    # fp16 hi/lo splits for scoring
    c_hi = m2c.astype(np.float16)
    c_lo = (m2c - c_hi.astype(np.float32)).astype(np.float16)
    e_hi = esq.astype(np.float16)
    e_lo = (esq - e_hi.astype(np.float64)).astype(np.float16)
    ra = np.concatenate([c_hi, e_hi[None, :], e_lo[None, :]], axis=0)  # [66,256]
    rc = c_lo                                                          # [64,256]

    iota1 = np.broadcast_to(
        (np.arange(256, dtype=np.float32) + 1.0)[None, :], (128, 256)
    ).copy()                                                      # bf16-exact

    rst = np.zeros(9, np.float32)
    rst[0] = BIGF
    rst[1:] = -BIGF
    rstmask = np.broadcast_to(np.tile(rst, NTILE)[None, :],
                              (128, NTILE * 9)).copy()

    selqs = []
    for rowlevel, rowcode in _tile_maps():
        # slots: l -> idx_l, 8+l -> rhi_l, 16+l -> rlo_l, 24/25 -> ones
        sq = np.zeros((26, 128), np.float32)
        for p_ in range(128):
            l = rowlevel[p_]
            if l >= 0:
                k = float(rowcode[p_])
                sq[l, p_] = -2.0 * k
                sq[8 + l, p_] = 256.0
                sq[16 + l, p_] = 1.0
                b = k * k - 1.0
                b_hi = np.floor(b / 256.0) * 256.0
                sq[24, p_] = b_hi
                sq[25, p_] = b - b_hi
            else:
                sq[25, p_] = 3.0    # pad: g2' = 3 -> Relu(-3) = 0
        selqs.append(sq)

    # gather lhsT weights (zero rows where the onehot row isn't contracted)
    g01 = np.zeros((128, 128), np.float32)   # rows 96-127 used
    g01[96:98, 0:64] = cb[0:2]        # l0 -> out cols 0:64
    g01[98:102, 64:128] = cb[0:4]     # l1 -> out cols 64:128
    g23 = np.zeros((128, 128), np.float32)
    g23[102:110, 0:64] = cb[0:8]      # l2
    g23[110:126, 64:128] = cb[0:16]   # l3
    g45 = np.zeros((96, 128), np.float32)
    g45[0:32, 0:64] = cb[0:32]        # l4 (T1 rows 0-31)
    g45[32:96, 64:128] = cb[0:64]     # l5 (T1 rows 32-95)
    c67a = np.zeros((128, 128), np.float32)
    c67a[:, 0:64] = cb[0:128]                 # l6 -> out cols 0:64
    c67b = np.zeros((128, 128), np.float32)
    c67b[:, 64:128] = cb[0:128]               # l7 lo -> out cols 64:128
    c67c = np.zeros((128, 128), np.float32)
    c67c[:, 64:128] = cb[128:256]             # l7 hi -> out cols 64:128
    ident = np.eye(128, dtype=np.float32)
    return dict(
        ra=np.ascontiguousarray(ra), rc=np.ascontiguousarray(rc),
        iota1=_bf16(iota1), rstmask=rstmask,
        selq1=_bf16(selqs[0]), selq2=_bf16(selqs[1]),
        selq3=_bf16(selqs[2]), selq4=_bf16(selqs[3]),
        g01=_bf16(g01), g23=_bf16(g23), g45=_bf16(g45),
        c67a=_bf16(c67a), c67b=_bf16(c67b), c67c=_bf16(c67c),
        ident=_bf16(ident),
    )


def _split_x(x):
    """x [B, 64, HWTOK] f32 -> xhi [B, 66, HWTOK] f16 (ones rows 64-65),
    xlo [B, 64, HWTOK] f16."""
    xb = np.asarray(x, dtype=np.float32)
    n = xb.shape[0]
    xhi = np.empty((n, 66, HWTOK), np.float16)
    hi = xb.astype(np.float16)
    xhi[:, 0:64] = hi
    xhi[:, 64:66] = 1.0
    xlo = (xb - hi.astype(np.float32)).astype(np.float16)
    return np.ascontiguousarray(xhi), np.ascontiguousarray(xlo)


_CACHE = {}

# block b covers codes [blo, bhi)
_BLOCKS = [(0, 2)] + [(1 << b, 1 << (b + 1)) for b in range(1, 8)]


def _build(debug=False):
    key = "nc_dbg" if debug else "nc"
    if key in _CACHE:
        return _CACHE[key]
    from concourse import bass, bacc, tile, mybir

    f32 = mybir.dt.float32
    f16 = mybir.dt.float16
    bf16 = mybir.dt.bfloat16
    i32 = mybir.dt.int32
    Alu = mybir.AluOpType
    Act = mybir.ActivationFunctionType
    AxX = mybir.AxisListType.X

    nc = bacc.Bacc("TRN2", target_bir_lowering=False, debug=False,
                   num_devices=NCORES)
    xhi_d = nc.declare_dram_parameter("xhi", [IMGS, 66, HWTOK], f16,
                                      isOutput=False)
    xlo_d = nc.declare_dram_parameter("xlo", [IMGS, 64, HWTOK], f16,
                                      isOutput=False)
    consts_spec = [
        ("ra", [66, 256], f16), ("rc", [64, 256], f16),
        ("iota1", [128, 256], bf16), ("rstmask", [128, NTILE * 9], f32),
        ("selq1", [26, 128], bf16), ("selq2", [26, 128], bf16),
        ("selq3", [26, 128], bf16), ("selq4", [26, 128], bf16),
        ("g01", [128, 128], bf16), ("g23", [128, 128], bf16),
        ("g45", [96, 128], bf16),
        ("c67a", [128, 128], bf16), ("c67b", [128, 128], bf16),
        ("c67c", [128, 128], bf16), ("ident", [128, 128], bf16),
    ]
    cd = {name: nc.declare_dram_parameter(name, shp, dt, isOutput=False)
          for name, shp, dt in consts_spec}
    out_d = nc.declare_dram_parameter("out", [IMGS, 4, 128, HWTOK], bf16,
                                      isOutput=True)
    dbg = {}
    if debug:
        for nm, shp, dt in [
            ("dbg_keybuf", [128, NTILE, 256], f32),
            ("dbg_bmx", [128, NTILE * 9], f32),
            ("dbg_runmin", [128, NTILE * 9], f32),
            ("dbg_bidx1", [128, NTILE * 8], f32),
            ("dbg_scr8", [128, 34, NTILE], f32),
            ("dbg_idxq", [128, NTILE, 32], bf16),
            ("dbg_R", [32, 512], bf16),
            ("dbg_oht", [128, 2048], bf16),
        ]:
            dbg[nm] = nc.declare_dram_parameter(nm, shp, dt, isOutput=True)

    with tile.TileContext(nc) as tc:
        with (
            tc.tile_pool(name="const", bufs=1) as cpool,
            tc.tile_pool(name="xp", bufs=2) as xpool,
            tc.tile_pool(name="kb", bufs=2) as kbpool,
            tc.tile_pool(name="eqp", bufs=1) as eqpool,
            tc.tile_pool(name="small", bufs=2) as smpool,
            tc.tile_pool(name="ohp", bufs=2) as ohpool,
            tc.tile_pool(name="outp", bufs=2) as outpool,
            tc.tile_pool(name="kps", bufs=2, space="PSUM") as kpsum,
            tc.tile_pool(name="gqs", bufs=1, space="PSUM") as gqpsum,
            tc.tile_pool(name="gos", bufs=2, space="PSUM") as gopsum,
        ):
            cs = {}
            for name, shp, dt in consts_spec:
                t = cpool.tile(shp, dt, tag=name, name=name)
                nc.sync.dma_start(out=t[:], in_=cd[name][:])
                cs[name] = t

            xts = {}
            kbs = {}

            def emit_score(img):
                xh = xpool.tile([66, HWTOK], f16, tag="xh", name="xh")
                nc.sync.dma_start(out=xh[:], in_=xhi_d[img])
                xl = xpool.tile([64, HWTOK], f16, tag="xl", name="xl")
                nc.sync.dma_start(out=xl[:], in_=xlo_d[img])
                keybuf = kbpool.tile([128, NTILE, 256], f32, tag="kb",
                                     name="keybuf")
                for tt in range(NTILE // 2):
                    kp = kpsum.tile([128, 512], f32, tag="kp", name="kp")
                    for j in range(2):
                        t = 2 * tt + j
                        sl = slice(t * 128, (t + 1) * 128)
                        dst = kp[:, j * 256:(j + 1) * 256]
                        nc.tensor.matmul(dst, xh[:, sl], cs["ra"][:],
                                         start=True, stop=False)
                        nc.tensor.matmul(dst, xl[:, sl], cs["ra"][0:64, :],
                                         start=False, stop=False)
                        nc.tensor.matmul(dst, xh[0:64, sl], cs["rc"][:],
                                         start=False, stop=True)
                    dst = keybuf[:, 2 * tt:2 * tt + 2, :]
                    if tt % 2 == 0:
                        nc.scalar.copy(dst, kp[:])
                    else:
                        nc.vector.tensor_copy(dst, kp[:])
                xts[img] = (xh, xl)
                kbs[img] = keybuf

            def emit_argmin(img):
                keybuf = kbs[img]
                bmx = smpool.tile([128, NTILE * 9], f32, tag="bmx",
                                  name="bmx")
                nc.gpsimd.memset(bmx[:, 0::9], BIGF)
                for b, (blo, bhi) in enumerate(_BLOCKS):
                    nc.vector.tensor_reduce(
                        out=bmx[:, 1 + b::9], in_=keybuf[:, :, blo:bhi],
                        axis=AxX, op=Alu.min,
                    )
                runmin = smpool.tile([128, NTILE * 9], f32, tag="runmin",
                                     name="runmin")
                nc.vector.tensor_tensor_scan(
                    out=runmin[:], data0=bmx[:], data1=cs["rstmask"][:],
                    initial=BIGF, op0=Alu.min, op1=Alu.max,
                )
                eqs = eqpool.tile([128, NTILE, 256], bf16, tag="eqs",
                                  name="eqs")
                for b, (blo, bhi) in enumerate(_BLOCKS):
                    h = bhi - blo
                    bmb = bmx[:, 1 + b::9].unsqueeze(2).to_broadcast(
                        [128, NTILE, h])
                    nc.vector.tensor_tensor(
                        out=eqs[:, :, blo:bhi], in0=keybuf[:, :, blo:bhi],
                        in1=bmb, op=Alu.is_equal,
                    )
                # mask *= (code+1), bf16-exact
                iot = cs["iota1"].unsqueeze(1)
                nc.vector.tensor_tensor(
                    out=eqs[:, :, 0:128], in0=eqs[:, :, 0:128],
                    in1=iot[:, :, 0:128].to_broadcast([128, NTILE, 128]),
                    op=Alu.mult,
                )
                nc.vector.tensor_tensor(
                    out=eqs[:, :, 128:256], in0=eqs[:, :, 128:256],
                    in1=iot[:, :, 128:256].to_broadcast([128, NTILE, 128]),
                    op=Alu.mult,
                )
                bidx1 = smpool.tile([128, NTILE * 8], f32, tag="bidx1",
                                    name="bidx1")
                for b, (blo, bhi) in enumerate(_BLOCKS):
                    nc.vector.tensor_reduce(
                        out=bidx1[:, b::8], in_=eqs[:, :, blo:bhi],
                        axis=AxX, op=Alu.max,
                    )
                # per-level running argmin (1-based) -> scr8 slots 0..7
                scr8 = smpool.tile([128, 34, NTILE], f32, tag="scr8",
                                   name="scr8")
                nc.vector.tensor_copy(scr8[:, 0, :], bidx1[:, 0::8])
                for l in range(1, NLVL):
                    mask = smpool.tile([128, NTILE], i32, tag="mask",
                                       name="mask")
                    nc.vector.tensor_tensor(out=mask[:],
                                            in0=bmx[:, 1 + l::9],
                                            in1=runmin[:, l::9],
                                            op=Alu.is_lt)
                    nc.vector.tensor_copy(scr8[:, l, :], scr8[:, l - 1, :])
                    nc.vector.copy_predicated(scr8[:, l, :], mask[:],
                                              bidx1[:, l::8])
                # 1-based -> 0-based; squares; rhi/rlo split; ones rows
                nc.gpsimd.tensor_scalar(
                    out=scr8[:, 0:8, :], in0=scr8[:, 0:8, :],
                    scalar1=-1.0, scalar2=None, op0=Alu.add,
                )
                nc.gpsimd.memset(scr8[:, 24:26, :], 1.0)
                nc.gpsimd.tensor_tensor(
                    out=scr8[:, 26:34, :], in0=scr8[:, 0:8, :],
                    in1=scr8[:, 0:8, :], op=Alu.mult,
                )
                nc.gpsimd.tensor_scalar(
                    out=scr8[:, 8:16, :], in0=scr8[:, 26:34, :],
                    scalar1=1.0 / 256.0, scalar2=MAGIC, op0=Alu.mult,
                    op1=Alu.add,
                )
                nc.gpsimd.tensor_scalar(
                    out=scr8[:, 8:16, :], in0=scr8[:, 8:16, :],
                    scalar1=-MAGIC, scalar2=None, op0=Alu.add,
                )
                nc.gpsimd.tensor_scalar(
                    out=scr8[:, 16:24, :], in0=scr8[:, 8:16, :],
                    scalar1=256.0, scalar2=None, op0=Alu.mult,
                )
                nc.gpsimd.tensor_tensor(
                    out=scr8[:, 16:24, :], in0=scr8[:, 26:34, :],
                    in1=scr8[:, 16:24, :], op=Alu.subtract,
                )
                idxq = smpool.tile([128, NTILE, 32], bf16, tag="idxq",
                                   name="idxq")
                nc.gpsimd.tensor_copy(idxq[:],
                                      scr8[:, 0:32, :].transpose([0, 2, 1]))
                if debug and img == 0:
                    nc.sync.dma_start(out=dbg["dbg_keybuf"][:], in_=keybuf[:])
                    nc.sync.dma_start(out=dbg["dbg_bmx"][:], in_=bmx[:])
                    nc.sync.dma_start(out=dbg["dbg_runmin"][:], in_=runmin[:])
                    nc.sync.dma_start(out=dbg["dbg_bidx1"][:], in_=bidx1[:])
                    nc.sync.dma_start(out=dbg["dbg_scr8"][:], in_=scr8[:])
                    nc.sync.dma_start(out=dbg["dbg_idxq"][:], in_=idxq[:])
                kbs[img] = None
                return idxq

            def emit_spans(img, idxq):
                stgs = []
                for pi in range(4):
                    stg = outpool.tile([128, 2048], bf16, tag=f"st{pi}",
                                       name=f"st{pi}")
                    stgs.append(stg)
                for s in range(NSPAN):
                    half = s // 4
                    so = (s % 4) * 512
                    tp = gopsum.tile([128, 128], bf16, tag="go", name="tp")
                    nc.tensor.transpose(
                        tp[:], idxq[:, 4 * s:4 * s + 4, :],
                        cs["ident"][:],
                    )
                    R = smpool.tile([32, 512], bf16, tag="R", name="R")
                    for t in range(4):
                        dst = R[0:26, t * 128:(t + 1) * 128]
                        src = tp[32 * t:32 * t + 26, :]
                        if t % 2 == 0:
                            nc.vector.tensor_copy(dst, src)
                        else:
                            nc.scalar.copy(dst, src)
                    gq = gqpsum.tile([128, 2048], f32, tag="gq", name="gq")
                    # T3, T4 first so Relu-B overlaps T1/T2 selectors
                    for X, selq in ((2, "selq3"), (3, "selq4"),
                                    (0, "selq1"), (1, "selq2")):
                        nc.tensor.matmul(
                            gq[:, X * 512:(X + 1) * 512],
                            cs[selq][:], R[0:26, :], start=True, stop=True,
                        )
                    oht = ohpool.tile([128, 2048], bf16, tag="oht",
                                      name="oht")
                    nc.scalar.activation(oht[:, 1024:2048],
                                         gq[:, 1024:2048], Act.Relu,
                                         bias=0.0, scale=-1.0)
                    nc.scalar.activation(oht[:, 0:1024], gq[:, 0:1024],
                                         Act.Relu, bias=0.0, scale=-1.0)
                    if debug and img == 0 and s == 0:
                        nc.sync.dma_start(out=dbg["dbg_R"][:], in_=R[:])
                        nc.sync.dma_start(out=dbg["dbg_oht"][:], in_=oht[:])

                    ps01 = gopsum.tile([128, 512], f32, tag="go",
                                       name="ps01")
                    nc.tensor.matmul(ps01[:], cs["g01"][96:128, :],
                                     oht[96:128, 0:512],
                                     start=True, stop=True,
                                     tile_position=(96, 0))
                    nc.scalar.copy(stgs[0][:, so:so + 512], ps01[:])
                    ps23 = gopsum.tile([128, 512], f32, tag="go",
                                       name="ps23")
                    nc.tensor.matmul(ps23[:], cs["g23"][96:128, :],
                                     oht[96:128, 0:512],
                                     start=True, stop=True,
                                     tile_position=(96, 0))
                    nc.vector.tensor_copy(stgs[1][:, so:so + 512], ps23[:])
                    ps45 = gopsum.tile([128, 512], f32, tag="go",
                                       name="ps45")
                    nc.tensor.matmul(ps45[:], cs["g45"][:],
                                     oht[0:96, 0:512],
                                     start=True, stop=True)
                    nc.scalar.copy(stgs[2][:, so:so + 512], ps45[:])
                    ps67 = gopsum.tile([128, 512], f32, tag="go",
                                       name="ps67")
                    nc.tensor.matmul(ps67[:], cs["c67a"][:],
                                     oht[:, 512:1024],
                                     start=True, stop=False)
                    nc.tensor.matmul(ps67[:], cs["c67b"][:],
                                     oht[:, 1024:1536],
                                     start=False, stop=False)
                    nc.tensor.matmul(ps67[:], cs["c67c"][:],
                                     oht[:, 1536:2048],
                                     start=False, stop=True)
                    nc.vector.tensor_copy(stgs[3][:, so:so + 512], ps67[:])

                    if s % 4 == 3:
                        for pi in range(4):
                            nc.sync.dma_start(
                                out=out_d[img, pi, :,
                                          half * 2048:(half + 1) * 2048],
                                in_=stgs[pi][:])
                        if s == 3:
                            stgs = []
                            for pi in range(4):
                                stg = outpool.tile([128, 2048], bf16,
                                                   tag=f"st{pi}",
                                                   name=f"st{pi}")
                                stgs.append(stg)

            emit_score(0)
            for img in range(IMGS):
                idxq = emit_argmin(img)
                if img + 1 < IMGS:
                    emit_score(img + 1)
                emit_spans(img, idxq)
    nc.compile()
    _CACHE[key] = nc
    return nc


def _prepare_in_maps(input_data, codebook):
    x = np.ascontiguousarray(np.asarray(input_data, dtype=np.float32))
    assert x.shape == (B, C, H, W)
    consts = _host_consts(np.asarray(codebook, dtype=np.float32))
    xhi, xlo = _split_x(x.reshape(B, 64, HWTOK))
    in_maps = []
    for core in range(NCORES):
        sl = slice(core * IMGS, (core + 1) * IMGS)
        m = {"xhi": np.ascontiguousarray(xhi[sl]),
             "xlo": np.ascontiguousarray(xlo[sl])}
        m.update(consts)
        in_maps.append(m)
    return in_maps


def kernel(input_data, codebook, previous_active_vectors=None,
           num_active_vectors=256, **_):
    from concourse.bass_utils import run_bass_kernel_spmd

    nc = _build()
    in_maps = _prepare_in_maps(input_data, codebook)
    res = run_bass_kernel_spmd(nc, in_maps, core_ids=list(range(NCORES)))
    # per core: [IMGS, 4 lvl-pairs, 128 (=2 lvl x 64 ch), HWTOK] bf16
    outs = [np.asarray(res.results[i]["out"]) for i in range(NCORES)]
    full = np.stack(outs, axis=0).astype(np.float32)
    full = full.reshape(NCORES, IMGS, 4, 2, 64, HWTOK)
    full = full.transpose(2, 3, 0, 1, 4, 5)   # [pair, half, core, img, ch, t]
    return np.ascontiguousarray(full.reshape(NLVL, B, C, H, W))


# revision 10
# speedup vs baseline: 1.0756x; 1.0756x over previous
"""AdaptiveVectorQuantizer Trainium2 kernel (8 NeuronCores, data-parallel).

v2: PE-lean rewrite of the baseline.
 - Scoring: fp16 hi/lo split of x and -2*cb.T -> 3 bf16-rate matmuls
   (1 cyc/col) instead of fp32 LOW_HIGH (4 cyc/col).  score = e_sq - 2 x.e
   exact to ~1e-8 (fp16 products are exact in fp32 PSUM accumulate).
 - Argmin: per-block min-reduce -> prefix-min via tensor_tensor_scan
   (reset-slot trick), eq-mask (fp32, split DVE/GPSIMD), mask * (code+1)
   in bf16 (exact ints <= 256), max-reduce extract (first-tie goes to the
   larger index - rare, within tolerance), select-chain for per-level idx.
 - Selector/gather matmuls all bf16 (operands are exact small ints or
   codebook values that round to bf16 output anyway).  The Relu bias
   (code^2-1 split hi/lo) rides in the selector matmul via two ones-rows,
   so one activation (scale=-1, bias=0) thresholds a whole [128,2048] gq.
 - Gather psum drained by per-span copies split ACT/DVE; output staged
   bf16, DMA'd per half-image.

Combined-onehot tiles (rows must be 32-aligned for engine slicing):
  T1: rows 0-31 l4 | 32-95 l5 | 96-125 l0..l3 | 126-127 pad
  T2: l6 codes 0-127.  T3: l7 codes 0-127.  T4: l7 codes 128-255.
"""

import sys
import numpy as np

sys.path.insert(0, "/opt/trn_rl_repo")

B, C, H, W = 32, 64, 64, 64
P, D = 256, 64
NCORES = 8
IMGS = B // NCORES            # 4 images per core
HWTOK = H * W                 # 4096 tokens per image
NTILE = HWTOK // 128          # 32
NSPAN = HWTOK // 512          # 8
NLVL = 8
BIGF = 1.0e30
MAGIC = float(2 ** 23)

# T1 row layout: rows 0-31: l4; 32-95: l5; 96-125: l0..l3; 126-127 pad
_T1_ROWS = [(4, j) for j in range(32)]
_T1_ROWS += [(5, j) for j in range(64)]
for lvl, k in ((0, 2), (1, 4), (2, 8), (3, 16)):
    _T1_ROWS += [(lvl, j) for j in range(k)]
_T1_ROWS += [(-1, 0), (-1, 0)]                     # rows 126-127 pad
assert len(_T1_ROWS) == 128


def _tile_maps():
    t1l = np.array([r[0] for r in _T1_ROWS])
    t1c = np.array([r[1] for r in _T1_ROWS])
    return [
        (t1l, t1c),
        (np.full(128, 6), np.arange(128)),            # T2: level 6
        (np.full(128, 7), np.arange(128)),            # T3: level 7 lo
        (np.full(128, 7), np.arange(128) + 128),      # T4: level 7 hi
    ]


def _bf16(a):
    import ml_dtypes
    return np.asarray(a, dtype=np.float32).astype(ml_dtypes.bfloat16)


def _host_consts(codebook):
    cb = np.ascontiguousarray(codebook, dtype=np.float32)         # [256, 64]
    esq = np.sum(cb.astype(np.float64) * cb, axis=1)              # [256]
    m2c = (-2.0 * cb.T).astype(np.float32)                        # [64, 256]

    # fp16 hi/lo splits for scoring
    c_hi = m2c.astype(np.float16)
    c_lo = (m2c - c_hi.astype(np.float32)).astype(np.float16)
    e_hi = esq.astype(np.float16)
    e_lo = (esq - e_hi.astype(np.float64)).astype(np.float16)
    ra = np.concatenate([c_hi, e_hi[None, :], e_lo[None, :]], axis=0)  # [66,256]
    rc = c_lo                                                          # [64,256]

    iota1 = np.broadcast_to(
        (np.arange(256, dtype=np.float32) + 1.0)[None, :], (128, 256)
    ).copy()                                                      # bf16-exact

    rst = np.zeros(9, np.float32)
    rst[0] = BIGF
    rst[1:] = -BIGF
    rstmask = np.broadcast_to(np.tile(rst, NTILE)[None, :],
                              (128, NTILE * 9)).copy()

    selqs = []
    for rowlevel, rowcode in _tile_maps():
        # slots: l -> idx_l, 8+l -> rhi_l, 16+l -> rlo_l, 24/25 -> ones
        sq = np.zeros((26, 128), np.float32)
        for p_ in range(128):
            l = rowlevel[p_]
            if l >= 0:
                k = float(rowcode[p_])
                sq[l, p_] = -2.0 * k
                sq[8 + l, p_] = 256.0
                sq[16 + l, p_] = 1.0
                b = k * k - 1.0
                b_hi = np.floor(b / 256.0) * 256.0
                sq[24, p_] = b_hi
                sq[25, p_] = b - b_hi
            else:
                sq[25, p_] = 3.0    # pad: g2' = 3 -> Relu(-3) = 0
        selqs.append(sq)

    # gather lhsT weights (zero rows where the onehot row isn't contracted)
    g01 = np.zeros((128, 128), np.float32)   # rows 96-127 used
    g01[96:98, 0:64] = cb[0:2]        # l0 -> out cols 0:64
    g01[98:102, 64:128] = cb[0:4]     # l1 -> out cols 64:128
    g23 = np.zeros((128, 128), np.float32)
    g23[102:110, 0:64] = cb[0:8]      # l2
    g23[110:126, 64:128] = cb[0:16]   # l3
    g45 = np.zeros((96, 128), np.float32)
    g45[0:32, 0:64] = cb[0:32]        # l4 (T1 rows 0-31)
    g45[32:96, 64:128] = cb[0:64]     # l5 (T1 rows 32-95)
    c67a = np.zeros((128, 128), np.float32)
    c67a[:, 0:64] = cb[0:128]                 # l6 -> out cols 0:64
    c67b = np.zeros((128, 128), np.float32)
    c67b[:, 64:128] = cb[0:128]               # l7 lo -> out cols 64:128
    c67c = np.zeros((128, 128), np.float32)
    c67c[:, 64:128] = cb[128:256]             # l7 hi -> out cols 64:128
    ident = np.eye(128, dtype=np.float32)
    return dict(
        ra=np.ascontiguousarray(ra), rc=np.ascontiguousarray(rc),
        iota1=_bf16(iota1), rstmask=rstmask,
        selq1=_bf16(selqs[0]), selq2=_bf16(selqs[1]),
        selq3=_bf16(selqs[2]), selq4=_bf16(selqs[3]),
        g01=_bf16(g01), g23=_bf16(g23), g45=_bf16(g45),
        c67a=_bf16(c67a), c67b=_bf16(c67b), c67c=_bf16(c67c),
        ident=_bf16(ident),
    )


def _split_x(x):
    """x [B, 64, HWTOK] f32 -> xhi [B, 66, HWTOK] f16 (ones rows 64-65),
    xlo [B, 64, HWTOK] f16."""
    xb = np.asarray(x, dtype=np.float32)
    n = xb.shape[0]
    xhi = np.empty((n, 66, HWTOK), np.float16)
    hi = xb.astype(np.float16)
    xhi[:, 0:64] = hi
    xhi[:, 64:66] = 1.0
    xlo = (xb - hi.astype(np.float32)).astype(np.float16)
    return np.ascontiguousarray(xhi), np.ascontiguousarray(xlo)


_CACHE = {}

# block b covers codes [blo, bhi)
_BLOCKS = [(0, 2)] + [(1 << b, 1 << (b + 1)) for b in range(1, 8)]


def _build(debug=False):
    key = "nc_dbg" if debug else "nc"
    if key in _CACHE:
        return _CACHE[key]
    from concourse import bass, bacc, tile, mybir

    f32 = mybir.dt.float32
    f16 = mybir.dt.float16
    bf16 = mybir.dt.bfloat16
    i32 = mybir.dt.int32
    Alu = mybir.AluOpType
    Act = mybir.ActivationFunctionType
    AxX = mybir.AxisListType.X

    nc = bacc.Bacc("TRN2", target_bir_lowering=False, debug=False,
                   num_devices=NCORES)
    xhi_d = nc.declare_dram_parameter("xhi", [IMGS, 66, HWTOK], f16,
                                      isOutput=False)
    xlo_d = nc.declare_dram_parameter("xlo", [IMGS, 64, HWTOK], f16,
                                      isOutput=False)
    consts_spec = [
        ("ra", [66, 256], f16), ("rc", [64, 256], f16),
        ("iota1", [128, 256], bf16), ("rstmask", [128, NTILE * 9], f32),
        ("selq1", [26, 128], bf16), ("selq2", [26, 128], bf16),
        ("selq3", [26, 128], bf16), ("selq4", [26, 128], bf16),
        ("g01", [128, 128], bf16), ("g23", [128, 128], bf16),
        ("g45", [96, 128], bf16),
        ("c67a", [128, 128], bf16), ("c67b", [128, 128], bf16),
        ("c67c", [128, 128], bf16), ("ident", [128, 128], bf16),
    ]
    cd = {name: nc.declare_dram_parameter(name, shp, dt, isOutput=False)
          for name, shp, dt in consts_spec}
    out_d = nc.declare_dram_parameter("out", [IMGS, 4, 128, HWTOK], bf16,
                                      isOutput=True)
    dbg = {}
    if debug:
        for nm, shp, dt in [
            ("dbg_keybuf", [128, NTILE, 256], f32),
            ("dbg_bmx", [128, NTILE * 9], f32),
            ("dbg_runmin", [128, NTILE * 9], f32),
            ("dbg_bidx1", [128, NTILE * 8], f32),
            ("dbg_scr8", [128, 34, NTILE], f32),
            ("dbg_idxq", [128, NTILE, 32], bf16),
            ("dbg_R", [32, 512], bf16),
            ("dbg_oht", [128, 2048], bf16),
        ]:
            dbg[nm] = nc.declare_dram_parameter(nm, shp, dt, isOutput=True)

    with tile.TileContext(nc) as tc:
        with (
            tc.tile_pool(name="const", bufs=1) as cpool,
            tc.tile_pool(name="xp", bufs=2) as xpool,
            tc.tile_pool(name="kb", bufs=2) as kbpool,
            tc.tile_pool(name="eqp", bufs=1) as eqpool,
            tc.tile_pool(name="small", bufs=2) as smpool,
            tc.tile_pool(name="ohp", bufs=2) as ohpool,
            tc.tile_pool(name="outp", bufs=2) as outpool,
            tc.tile_pool(name="kps", bufs=2, space="PSUM") as kpsum,
            tc.tile_pool(name="gqs", bufs=1, space="PSUM") as gqpsum,
            tc.tile_pool(name="gos", bufs=2, space="PSUM") as gopsum,
        ):
            cs = {}
            for name, shp, dt in consts_spec:
                t = cpool.tile(shp, dt, tag=name, name=name)
                nc.sync.dma_start(out=t[:], in_=cd[name][:])
                cs[name] = t

            xts = {}
            kbs = {}

            def emit_score_dma(img):
                xh = xpool.tile([66, HWTOK], f16, tag="xh", name="xh")
                nc.sync.dma_start(out=xh[:], in_=xhi_d[img])
                xl = xpool.tile([64, HWTOK], f16, tag="xl", name="xl")
                nc.sync.dma_start(out=xl[:], in_=xlo_d[img])
                keybuf = kbpool.tile([128, NTILE, 256], f32, tag="kb",
                                     name="keybuf")
                xts[img] = (xh, xl)
                kbs[img] = keybuf

            def emit_score_group(img, tt):
                xh, xl = xts[img]
                keybuf = kbs[img]
                kp = kpsum.tile([128, 512], f32, tag="kp", name="kp")
                for j in range(2):
                    t = 2 * tt + j
                    sl = slice(t * 128, (t + 1) * 128)
                    dst = kp[:, j * 256:(j + 1) * 256]
                    nc.tensor.matmul(dst, xh[:, sl], cs["ra"][:],
                                     start=True, stop=False)
                    nc.tensor.matmul(dst, xl[:, sl], cs["ra"][0:64, :],
                                     start=False, stop=False)
                    nc.tensor.matmul(dst, xh[0:64, sl], cs["rc"][:],
                                     start=False, stop=True)
                nc.scalar.copy(keybuf[:, 2 * tt:2 * tt + 2, :], kp[:])

            def emit_argmin(img):
                keybuf = kbs[img]
                bmx = smpool.tile([128, NTILE * 9], f32, tag="bmx",
                                  name="bmx")
                nc.gpsimd.memset(bmx[:, 0::9], BIGF)
                for b, (blo, bhi) in enumerate(_BLOCKS):
                    nc.vector.tensor_reduce(
                        out=bmx[:, 1 + b::9], in_=keybuf[:, :, blo:bhi],
                        axis=AxX, op=Alu.min,
                    )
                runmin = smpool.tile([128, NTILE * 9], f32, tag="runmin",
                                     name="runmin")
                nc.vector.tensor_tensor_scan(
                    out=runmin[:], data0=bmx[:], data1=cs["rstmask"][:],
                    initial=BIGF, op0=Alu.min, op1=Alu.max,
                )
                eqs = eqpool.tile([128, NTILE, 256], bf16, tag="eqs",
                                  name="eqs")
                for b, (blo, bhi) in enumerate(_BLOCKS):
                    h = bhi - blo
                    bmb = bmx[:, 1 + b::9].unsqueeze(2).to_broadcast(
                        [128, NTILE, h])
                    nc.vector.tensor_tensor(
                        out=eqs[:, :, blo:bhi], in0=keybuf[:, :, blo:bhi],
                        in1=bmb, op=Alu.is_equal,
                    )
                # mask *= (code+1), bf16-exact
                iot = cs["iota1"].unsqueeze(1)
                nc.vector.tensor_tensor(
                    out=eqs[:, :, 0:128], in0=eqs[:, :, 0:128],
                    in1=iot[:, :, 0:128].to_broadcast([128, NTILE, 128]),
                    op=Alu.mult,
                )
                nc.vector.tensor_tensor(
                    out=eqs[:, :, 128:256], in0=eqs[:, :, 128:256],
                    in1=iot[:, :, 128:256].to_broadcast([128, NTILE, 128]),
                    op=Alu.mult,
                )
                bidx1 = smpool.tile([128, NTILE * 8], f32, tag="bidx1",
                                    name="bidx1")
                for b, (blo, bhi) in enumerate(_BLOCKS):
                    nc.vector.tensor_reduce(
                        out=bidx1[:, b::8], in_=eqs[:, :, blo:bhi],
                        axis=AxX, op=Alu.max,
                    )
                # per-level running argmin (1-based) -> scr8 slots 0..7
                scr8 = smpool.tile([128, 34, NTILE], f32, tag="scr8",
                                   name="scr8")
                nc.vector.tensor_copy(scr8[:, 0, :], bidx1[:, 0::8])
                for l in range(1, NLVL):
                    mask = smpool.tile([128, NTILE], i32, tag="mask",
                                       name="mask")
                    nc.vector.tensor_tensor(out=mask[:],
                                            in0=bmx[:, 1 + l::9],
                                            in1=runmin[:, l::9],
                                            op=Alu.is_lt)
                    nc.vector.tensor_copy(scr8[:, l, :], scr8[:, l - 1, :])
                    nc.vector.copy_predicated(scr8[:, l, :], mask[:],
                                              bidx1[:, l::8])
                # 1-based -> 0-based; squares; rhi/rlo split; ones rows
                nc.vector.tensor_scalar(
                    out=scr8[:, 0:8, :], in0=scr8[:, 0:8, :],
                    scalar1=-1.0, scalar2=None, op0=Alu.add,
                )
                nc.gpsimd.memset(scr8[:, 24:26, :], 1.0)
                nc.vector.tensor_tensor(
                    out=scr8[:, 26:34, :], in0=scr8[:, 0:8, :],
                    in1=scr8[:, 0:8, :], op=Alu.mult,
                )
                nc.vector.tensor_scalar(
                    out=scr8[:, 8:16, :], in0=scr8[:, 26:34, :],
                    scalar1=1.0 / 256.0, scalar2=MAGIC, op0=Alu.mult,
                    op1=Alu.add,
                )
                nc.vector.tensor_scalar(
                    out=scr8[:, 8:16, :], in0=scr8[:, 8:16, :],
                    scalar1=-MAGIC, scalar2=None, op0=Alu.add,
                )
                nc.vector.tensor_scalar(
                    out=scr8[:, 16:24, :], in0=scr8[:, 8:16, :],
                    scalar1=256.0, scalar2=None, op0=Alu.mult,
                )
                nc.vector.tensor_tensor(
                    out=scr8[:, 16:24, :], in0=scr8[:, 26:34, :],
                    in1=scr8[:, 16:24, :], op=Alu.subtract,
                )
                idxq = smpool.tile([128, NTILE, 32], bf16, tag="idxq",
                                   name="idxq")
                nc.vector.tensor_copy(idxq[:],
                                      scr8[:, 0:32, :].transpose([0, 2, 1]))
                if debug and img == 0:
                    nc.sync.dma_start(out=dbg["dbg_keybuf"][:], in_=keybuf[:])
                    nc.sync.dma_start(out=dbg["dbg_bmx"][:], in_=bmx[:])
                    nc.sync.dma_start(out=dbg["dbg_runmin"][:], in_=runmin[:])
                    nc.sync.dma_start(out=dbg["dbg_bidx1"][:], in_=bidx1[:])
                    nc.sync.dma_start(out=dbg["dbg_scr8"][:], in_=scr8[:])
                    nc.sync.dma_start(out=dbg["dbg_idxq"][:], in_=idxq[:])
                kbs[img] = None
                return idxq

            def emit_spans(img, idxq, feed):
                stgs = []
                for pi in range(4):
                    stg = outpool.tile([128, 2048], bf16, tag=f"st{pi}",
                                       name=f"st{pi}")
                    stgs.append(stg)
                for s in range(NSPAN):
                    half = s // 4
                    so = (s % 4) * 512
                    tp = gopsum.tile([128, 128], bf16, tag="go", name="tp")
                    nc.tensor.transpose(
                        tp[:], idxq[:, 4 * s:4 * s + 4, :],
                        cs["ident"][:],
                    )
                    feed(1)
                    R = smpool.tile([32, 512], bf16, tag="R", name="R")
                    for t in range(4):
                        dst = R[0:26, t * 128:(t + 1) * 128]
                        src = tp[32 * t:32 * t + 26, :]
                        if t % 2 == 0:
                            nc.vector.tensor_copy(dst, src)
                        else:
                            nc.scalar.copy(dst, src)
                    gq = gqpsum.tile([128, 2048], f32, tag="gq", name="gq")
                    # T3, T4 first so Relu-B overlaps T1/T2 selectors
                    for X, selq in ((2, "selq3"), (3, "selq4"),
                                    (0, "selq1"), (1, "selq2")):
                        nc.tensor.matmul(
                            gq[:, X * 512:(X + 1) * 512],
                            cs[selq][:], R[0:26, :], start=True, stop=True,
                        )
                    oht = ohpool.tile([128, 2048], bf16, tag="oht",
                                      name="oht")
                    nc.scalar.activation(oht[:, 1024:2048],
                                         gq[:, 1024:2048], Act.Relu,
                                         bias=0.0, scale=-1.0)
                    nc.scalar.activation(oht[:, 0:1024], gq[:, 0:1024],
                                         Act.Relu, bias=0.0, scale=-1.0)
                    if debug and img == 0 and s == 0:
                        nc.sync.dma_start(out=dbg["dbg_R"][:], in_=R[:])
                        nc.sync.dma_start(out=dbg["dbg_oht"][:], in_=oht[:])

                    ps01 = gopsum.tile([128, 512], f32, tag="go",
                                       name="ps01")
                    nc.tensor.matmul(ps01[:], cs["g01"][96:128, :],
                                     oht[96:128, 0:512],
                                     start=True, stop=True,
                                     tile_position=(96, 0))
                    nc.scalar.copy(stgs[0][:, so:so + 512], ps01[:])
                    ps23 = gopsum.tile([128, 512], f32, tag="go",
                                       name="ps23")
                    nc.tensor.matmul(ps23[:], cs["g23"][96:128, :],
                                     oht[96:128, 0:512],
                                     start=True, stop=True,
                                     tile_position=(96, 0))
                    nc.vector.tensor_copy(stgs[1][:, so:so + 512], ps23[:])
                    ps45 = gopsum.tile([128, 512], f32, tag="go",
                                       name="ps45")
                    nc.tensor.matmul(ps45[:], cs["g45"][:],
                                     oht[0:96, 0:512],
                                     start=True, stop=True)
                    nc.scalar.copy(stgs[2][:, so:so + 512], ps45[:])
                    ps67 = gopsum.tile([128, 512], f32, tag="go",
                                       name="ps67")
                    nc.tensor.matmul(ps67[:], cs["c67a"][:],
                                     oht[:, 512:1024],
                                     start=True, stop=False)
                    nc.tensor.matmul(ps67[:], cs["c67b"][:],
                                     oht[:, 1024:1536],
                                     start=False, stop=False)
                    nc.tensor.matmul(ps67[:], cs["c67c"][:],
                                     oht[:, 1536:2048],
                                     start=False, stop=True)
                    nc.vector.tensor_copy(stgs[3][:, so:so + 512], ps67[:])

                    if s % 4 == 3:
                        for pi in range(4):
                            nc.sync.dma_start(
                                out=out_d[img, pi, :,
                                          half * 2048:(half + 1) * 2048],
                                in_=stgs[pi][:])
                        if s == 3:
                            stgs = []
                            for pi in range(4):
                                stg = outpool.tile([128, 2048], bf16,
                                                   tag=f"st{pi}",
                                                   name=f"st{pi}")
                                stgs.append(stg)

            emit_score_dma(0)
            for tt in range(16):
                emit_score_group(0, tt)
            pend = []

            def feed(k):
                for _ in range(min(k, len(pend))):
                    nim, ntt = pend.pop(0)
                    emit_score_group(nim, ntt)

            for img in range(IMGS):
                if img + 1 < IMGS:
                    emit_score_dma(img + 1)
                    pend = [(img + 1, tt) for tt in range(16)]
                else:
                    pend = []
                idxq = emit_argmin(img)
                feed(8)
                emit_spans(img, idxq, feed)
                feed(16)
    nc.compile()
    _CACHE[key] = nc
    return nc


def _prepare_in_maps(input_data, codebook):
    x = np.ascontiguousarray(np.asarray(input_data, dtype=np.float32))
    assert x.shape == (B, C, H, W)
    consts = _host_consts(np.asarray(codebook, dtype=np.float32))
    xhi, xlo = _split_x(x.reshape(B, 64, HWTOK))
    in_maps = []
    for core in range(NCORES):
        sl = slice(core * IMGS, (core + 1) * IMGS)
        m = {"xhi": np.ascontiguousarray(xhi[sl]),
             "xlo": np.ascontiguousarray(xlo[sl])}
        m.update(consts)
        in_maps.append(m)
    return in_maps


def kernel(input_data, codebook, previous_active_vectors=None,
           num_active_vectors=256, **_):
    from concourse.bass_utils import run_bass_kernel_spmd

    nc = _build()
    in_maps = _prepare_in_maps(input_data, codebook)
    res = run_bass_kernel_spmd(nc, in_maps, core_ids=list(range(NCORES)))
    # per core: [IMGS, 4 lvl-pairs, 128 (=2 lvl x 64 ch), HWTOK] bf16
    outs = [np.asarray(res.results[i]["out"]) for i in range(NCORES)]
    full = np.stack(outs, axis=0).astype(np.float32)
    full = full.reshape(NCORES, IMGS, 4, 2, 64, HWTOK)
    full = full.transpose(2, 3, 0, 1, 4, 5)   # [pair, half, core, img, ch, t]
    return np.ascontiguousarray(full.reshape(NLVL, B, C, H, W))


# revision 13
# speedup vs baseline: 1.2233x; 1.1373x over previous
"""AdaptiveVectorQuantizer Trainium2 kernel (8 NeuronCores, data-parallel).

v2: PE-lean rewrite of the baseline.
 - Scoring: fp16 hi/lo split of x and -2*cb.T -> 3 bf16-rate matmuls
   (1 cyc/col) instead of fp32 LOW_HIGH (4 cyc/col).  score = e_sq - 2 x.e
   exact to ~1e-8 (fp16 products are exact in fp32 PSUM accumulate).
 - Argmin: per-block min-reduce -> prefix-min via tensor_tensor_scan
   (reset-slot trick), eq-mask (fp32, split DVE/GPSIMD), mask * (code+1)
   in bf16 (exact ints <= 256), max-reduce extract (first-tie goes to the
   larger index - rare, within tolerance), select-chain for per-level idx.
 - Selector/gather matmuls all bf16 (operands are exact small ints or
   codebook values that round to bf16 output anyway).  The Relu bias
   (code^2-1 split hi/lo) rides in the selector matmul via two ones-rows,
   so one activation (scale=-1, bias=0) thresholds a whole [128,2048] gq.
 - Gather psum drained by per-span copies split ACT/DVE; output staged
   bf16, DMA'd per half-image.

Combined-onehot tiles (rows must be 32-aligned for engine slicing):
  T1: rows 0-31 l4 | 32-95 l5 | 96-125 l0..l3 | 126-127 pad
  T2: l6 codes 0-127.  T3: l7 codes 0-127.  T4: l7 codes 128-255.
"""

import sys
import numpy as np

sys.path.insert(0, "/opt/trn_rl_repo")

B, C, H, W = 32, 64, 64, 64
P, D = 256, 64
NCORES = 8
IMGS = B // NCORES            # 4 images per core
HWTOK = H * W                 # 4096 tokens per image
NTILE = HWTOK // 128          # 32
NSPAN = HWTOK // 512          # 8
NLVL = 8
BIGF = 1.0e30
MAGIC = float(2 ** 23)

# T1 row layout: rows 0-31: l4; 32-95: l5; 96-125: l0..l3; 126-127 pad
_T1_ROWS = [(4, j) for j in range(32)]
_T1_ROWS += [(5, j) for j in range(64)]
for lvl, k in ((0, 2), (1, 4), (2, 8), (3, 16)):
    _T1_ROWS += [(lvl, j) for j in range(k)]
_T1_ROWS += [(-1, 0), (-1, 0)]                     # rows 126-127 pad
assert len(_T1_ROWS) == 128


def _tile_maps():
    t1l = np.array([r[0] for r in _T1_ROWS])
    t1c = np.array([r[1] for r in _T1_ROWS])
    return [
        (t1l, t1c),
        (np.full(128, 6), np.arange(128)),            # T2: level 6
        (np.full(128, 7), np.arange(128)),            # T3: level 7 lo
        (np.full(128, 7), np.arange(128) + 128),      # T4: level 7 hi
    ]


def _bf16(a):
    import ml_dtypes
    return np.asarray(a, dtype=np.float32).astype(ml_dtypes.bfloat16)


def _host_consts(codebook):
    cb = np.ascontiguousarray(codebook, dtype=np.float32)         # [256, 64]
    esq = np.sum(cb.astype(np.float64) * cb, axis=1)              # [256]
    m2c = (-2.0 * cb.T).astype(np.float32)                        # [64, 256]

    cbt2e = np.concatenate([m2c, esq.astype(np.float32)[None, :]],
                           axis=0)                                # [65, 256]
    onesrow = np.ones((1, HWTOK), np.float32)

    iota1 = np.broadcast_to(
        (np.arange(256, dtype=np.float32) + 1.0)[None, :], (128, 256)
    ).copy()                                                      # bf16-exact

    rst = np.zeros(9, np.float32)
    rst[0] = BIGF
    rst[1:] = -BIGF
    rstmask = np.broadcast_to(np.tile(rst, NTILE)[None, :],
                              (128, NTILE * 9)).copy()

    selqs = []
    for rowlevel, rowcode in _tile_maps():
        # slots: l -> idx_l, 8+l -> rhi_l, 16+l -> rlo_l, 24/25 -> ones
        sq = np.zeros((26, 128), np.float32)
        for p_ in range(128):
            l = rowlevel[p_]
            if l >= 0:
                k = float(rowcode[p_])
                sq[l, p_] = -2.0 * k
                sq[8 + l, p_] = 256.0
                sq[16 + l, p_] = 1.0
                b = k * k - 1.0
                b_hi = np.floor(b / 256.0) * 256.0
                sq[24, p_] = b_hi
                sq[25, p_] = b - b_hi
            else:
                sq[25, p_] = 3.0    # pad: g2' = 3 -> Relu(-3) = 0
        selqs.append(sq)

    # gather lhsT weights (zero rows where the onehot row isn't contracted)
    g01 = np.zeros((128, 128), np.float32)   # rows 96-127 used
    g01[96:98, 0:64] = cb[0:2]        # l0 -> out cols 0:64
    g01[98:102, 64:128] = cb[0:4]     # l1 -> out cols 64:128
    g23 = np.zeros((128, 128), np.float32)
    g23[102:110, 0:64] = cb[0:8]      # l2
    g23[110:126, 64:128] = cb[0:16]   # l3
    g45 = np.zeros((96, 128), np.float32)
    g45[0:32, 0:64] = cb[0:32]        # l4 (T1 rows 0-31)
    g45[32:96, 64:128] = cb[0:64]     # l5 (T1 rows 32-95)
    c67a = np.zeros((128, 128), np.float32)
    c67a[:, 0:64] = cb[0:128]                 # l6 -> out cols 0:64
    c67b = np.zeros((128, 128), np.float32)
    c67b[:, 64:128] = cb[0:128]               # l7 lo -> out cols 64:128
    c67c = np.zeros((128, 128), np.float32)
    c67c[:, 64:128] = cb[128:256]             # l7 hi -> out cols 64:128
    ident = np.eye(128, dtype=np.float32)
    return dict(
        cbt2e=np.ascontiguousarray(cbt2e), onesrow=onesrow,
        iota1=_bf16(iota1), rstmask=rstmask,
        selq1=_bf16(selqs[0]), selq2=_bf16(selqs[1]),
        selq3=_bf16(selqs[2]), selq4=_bf16(selqs[3]),
        g01=_bf16(g01), g23=_bf16(g23), g45=_bf16(g45),
        c67a=_bf16(c67a), c67b=_bf16(c67b), c67c=_bf16(c67c),
        ident=_bf16(ident),
    )


_CACHE = {}

# block b covers codes [blo, bhi)
_BLOCKS = [(0, 2)] + [(1 << b, 1 << (b + 1)) for b in range(1, 8)]


def _build(debug=False):
    key = "nc_dbg" if debug else "nc"
    if key in _CACHE:
        return _CACHE[key]
    from concourse import bass, bacc, tile, mybir

    f32 = mybir.dt.float32
    f16 = mybir.dt.float16
    bf16 = mybir.dt.bfloat16
    i32 = mybir.dt.int32
    Alu = mybir.AluOpType
    Act = mybir.ActivationFunctionType
    AxX = mybir.AxisListType.X

    nc = bacc.Bacc("TRN2", target_bir_lowering=False, debug=False,
                   num_devices=NCORES)
    x_d = nc.declare_dram_parameter("x", [IMGS, 64, HWTOK], f32,
                                    isOutput=False)
    consts_spec = [
        ("cbt2e", [65, 256], f32), ("onesrow", [1, HWTOK], f32),
        ("iota1", [128, 256], bf16), ("rstmask", [128, NTILE * 9], f32),
        ("selq1", [26, 128], bf16), ("selq2", [26, 128], bf16),
        ("selq3", [26, 128], bf16), ("selq4", [26, 128], bf16),
        ("g01", [128, 128], bf16), ("g23", [128, 128], bf16),
        ("g45", [96, 128], bf16),
        ("c67a", [128, 128], bf16), ("c67b", [128, 128], bf16),
        ("c67c", [128, 128], bf16), ("ident", [128, 128], bf16),
    ]
    cd = {name: nc.declare_dram_parameter(name, shp, dt, isOutput=False)
          for name, shp, dt in consts_spec}
    out_d = nc.declare_dram_parameter("out", [IMGS, 4, 128, HWTOK], bf16,
                                      isOutput=True)
    dbg = {}
    if debug:
        for nm, shp, dt in [
            ("dbg_keybuf", [128, NTILE, 256], f32),
            ("dbg_bmx", [128, NTILE * 9], f32),
            ("dbg_runmin", [128, NTILE * 9], f32),
            ("dbg_bidx1", [128, NTILE * 8], f32),
            ("dbg_scr8", [128, 34, NTILE], f32),
            ("dbg_idxq", [128, NTILE, 32], bf16),
            ("dbg_R", [32, 512], bf16),
            ("dbg_oht", [128, 2048], bf16),
        ]:
            dbg[nm] = nc.declare_dram_parameter(nm, shp, dt, isOutput=True)

    with tile.TileContext(nc) as tc:
        with (
            tc.tile_pool(name="const", bufs=1) as cpool,
            tc.tile_pool(name="xp", bufs=2) as xpool,
            tc.tile_pool(name="kb", bufs=2) as kbpool,
            tc.tile_pool(name="eqp", bufs=1) as eqpool,
            tc.tile_pool(name="small", bufs=2) as smpool,
            tc.tile_pool(name="ohp", bufs=2) as ohpool,
            tc.tile_pool(name="outp", bufs=2) as outpool,
            tc.tile_pool(name="kps", bufs=2, space="PSUM") as kpsum,
            tc.tile_pool(name="mix", bufs=3, space="PSUM") as mixpsum,
        ):
            cs = {}
            for name, shp, dt in consts_spec:
                t = cpool.tile(shp, dt, tag=name, name=name)
                nc.sync.dma_start(out=t[:], in_=cd[name][:])
                cs[name] = t

            xts = {}
            kbs = {}

            def emit_score_dma(img):
                xT = xpool.tile([65, HWTOK], f32, tag="xT", name="xT")
                nc.sync.dma_start(out=xT[0:64, :], in_=x_d[img])
                nc.sync.dma_start(out=xT[64:65, :], in_=cs["onesrow"][:])
                keybuf = kbpool.tile([128, NTILE, 256], f32, tag="kb",
                                     name="keybuf")
                xts[img] = xT
                kbs[img] = keybuf

            def emit_score_group(img, tt):
                xT = xts[img]
                keybuf = kbs[img]
                kp = kpsum.tile([128, 512], f32, tag="kp", name="kp")
                for j in range(2):
                    t = 2 * tt + j
                    nc.tensor.matmul(
                        kp[:, j * 256:(j + 1) * 256],
                        xT[:, t * 128:(t + 1) * 128], cs["cbt2e"][:],
                        start=True, stop=True,
                    )
                nc.scalar.copy(keybuf[:, 2 * tt:2 * tt + 2, :], kp[:])

            def emit_argmin(img):
                keybuf = kbs[img]
                bmx = smpool.tile([128, NTILE * 9], f32, tag="bmx",
                                  name="bmx")
                nc.gpsimd.memset(bmx[:, 0::9], BIGF)
                for b, (blo, bhi) in enumerate(_BLOCKS):
                    nc.vector.tensor_reduce(
                        out=bmx[:, 1 + b::9], in_=keybuf[:, :, blo:bhi],
                        axis=AxX, op=Alu.min,
                    )
                runmin = smpool.tile([128, NTILE * 9], f32, tag="runmin",
                                     name="runmin")
                nc.vector.tensor_tensor_scan(
                    out=runmin[:], data0=bmx[:], data1=cs["rstmask"][:],
                    initial=BIGF, op0=Alu.min, op1=Alu.max,
                )
                eqs = eqpool.tile([128, NTILE, 256], bf16, tag="eqs",
                                  name="eqs")
                for b, (blo, bhi) in enumerate(_BLOCKS):
                    h = bhi - blo
                    bmb = bmx[:, 1 + b::9].unsqueeze(2).to_broadcast(
                        [128, NTILE, h])
                    nc.vector.tensor_tensor(
                        out=eqs[:, :, blo:bhi], in0=keybuf[:, :, blo:bhi],
                        in1=bmb, op=Alu.is_equal,
                    )
                # mask *= (code+1), bf16-exact
                iot = cs["iota1"].unsqueeze(1)
                nc.vector.tensor_tensor(
                    out=eqs[:, :, 0:128], in0=eqs[:, :, 0:128],
                    in1=iot[:, :, 0:128].to_broadcast([128, NTILE, 128]),
                    op=Alu.mult,
                )
                nc.vector.tensor_tensor(
                    out=eqs[:, :, 128:256], in0=eqs[:, :, 128:256],
                    in1=iot[:, :, 128:256].to_broadcast([128, NTILE, 128]),
                    op=Alu.mult,
                )
                bidx1 = smpool.tile([128, NTILE * 8], f32, tag="bidx1",
                                    name="bidx1")
                for b, (blo, bhi) in enumerate(_BLOCKS):
                    nc.vector.tensor_reduce(
                        out=bidx1[:, b::8], in_=eqs[:, :, blo:bhi],
                        axis=AxX, op=Alu.max,
                    )
                # per-level running argmin (1-based) -> scr8 slots 0..7
                scr8 = smpool.tile([128, 34, NTILE], f32, tag="scr8",
                                   name="scr8")
                nc.vector.tensor_copy(scr8[:, 0, :], bidx1[:, 0::8])
                for l in range(1, NLVL):
                    mask = smpool.tile([128, NTILE], i32, tag="mask",
                                       name="mask")
                    nc.vector.tensor_tensor(out=mask[:],
                                            in0=bmx[:, 1 + l::9],
                                            in1=runmin[:, l::9],
                                            op=Alu.is_lt)
                    nc.vector.tensor_copy(scr8[:, l, :], scr8[:, l - 1, :])
                    nc.vector.copy_predicated(scr8[:, l, :], mask[:],
                                              bidx1[:, l::8])
                # 1-based -> 0-based; squares; rhi/rlo split; ones rows
                nc.vector.tensor_scalar(
                    out=scr8[:, 0:8, :], in0=scr8[:, 0:8, :],
                    scalar1=-1.0, scalar2=None, op0=Alu.add,
                )
                nc.gpsimd.memset(scr8[:, 24:26, :], 1.0)
                nc.vector.tensor_tensor(
                    out=scr8[:, 26:34, :], in0=scr8[:, 0:8, :],
                    in1=scr8[:, 0:8, :], op=Alu.mult,
                )
                nc.vector.tensor_scalar(
                    out=scr8[:, 8:16, :], in0=scr8[:, 26:34, :],
                    scalar1=1.0 / 256.0, scalar2=MAGIC, op0=Alu.mult,
                    op1=Alu.add,
                )
                nc.vector.tensor_scalar(
                    out=scr8[:, 8:16, :], in0=scr8[:, 8:16, :],
                    scalar1=-MAGIC, scalar2=None, op0=Alu.add,
                )
                nc.vector.tensor_scalar(
                    out=scr8[:, 16:24, :], in0=scr8[:, 8:16, :],
                    scalar1=256.0, scalar2=None, op0=Alu.mult,
                )
                nc.vector.tensor_tensor(
                    out=scr8[:, 16:24, :], in0=scr8[:, 26:34, :],
                    in1=scr8[:, 16:24, :], op=Alu.subtract,
                )
                idxq = smpool.tile([128, NTILE, 32], bf16, tag="idxq",
                                   name="idxq")
                nc.vector.tensor_copy(idxq[:],
                                      scr8[:, 0:32, :].transpose([0, 2, 1]))
                if debug and img == 0:
                    nc.sync.dma_start(out=dbg["dbg_keybuf"][:], in_=keybuf[:])
                    nc.sync.dma_start(out=dbg["dbg_bmx"][:], in_=bmx[:])
                    nc.sync.dma_start(out=dbg["dbg_runmin"][:], in_=runmin[:])
                    nc.sync.dma_start(out=dbg["dbg_bidx1"][:], in_=bidx1[:])
                    nc.sync.dma_start(out=dbg["dbg_scr8"][:], in_=scr8[:])
                    nc.sync.dma_start(out=dbg["dbg_idxq"][:], in_=idxq[:])
                kbs[img] = None
                return idxq

            def emit_spans(img, idxq, feed):
                stgs = []
                for pi in range(4):
                    stg = outpool.tile([128, 2048], bf16, tag=f"st{pi}",
                                       name=f"st{pi}")
                    stgs.append(stg)
                for s in range(NSPAN):
                    half = s // 4
                    so = (s % 4) * 512
                    tp = mixpsum.tile([128, 128], bf16, tag="mix", name="tp")
                    nc.tensor.transpose(
                        tp[:], idxq[:, 4 * s:4 * s + 4, :],
                        cs["ident"][:],
                    )
                    feed(1)
                    R = smpool.tile([32, 512], bf16, tag="R", name="R")
                    for t in range(4):
                        dst = R[0:26, t * 128:(t + 1) * 128]
                        src = tp[32 * t:32 * t + 26, :]
                        if t % 2 == 0:
                            nc.vector.tensor_copy(dst, src)
                        else:
                            nc.scalar.copy(dst, src)
                    # gqA = [T3 | T4], gqB = [T2-first | T1]
                    gqA = mixpsum.tile([128, 1024], f32, tag="mix",
                                       name="gqA")
                    nc.tensor.matmul(gqA[:, 0:512], cs["selq3"][:],
                                     R[0:26, :], start=True, stop=True)
                    nc.tensor.matmul(gqA[:, 512:1024], cs["selq4"][:],
                                     R[0:26, :], start=True, stop=True)
                    gqB = mixpsum.tile([128, 1024], f32, tag="mix",
                                       name="gqB")
                    nc.tensor.matmul(gqB[:, 512:1024], cs["selq2"][:],
                                     R[0:26, :], start=True, stop=True)
                    nc.tensor.matmul(gqB[:, 0:512], cs["selq1"][:],
                                     R[0:26, :], start=True, stop=True)
                    oht = ohpool.tile([128, 2048], bf16, tag="oht",
                                      name="oht")
                    nc.scalar.activation(oht[:, 1024:2048], gqA[:],
                                         Act.Relu, bias=0.0, scale=-1.0)
                    nc.scalar.activation(oht[:, 512:1024],
                                         gqB[:, 512:1024], Act.Relu,
                                         bias=0.0, scale=-1.0)
                    nc.scalar.activation(oht[:, 0:512], gqB[:, 0:512],
                                         Act.Relu, bias=0.0, scale=-1.0)
                    if debug and img == 0 and s == 0:
                        nc.sync.dma_start(out=dbg["dbg_R"][:], in_=R[:])
                        nc.sync.dma_start(out=dbg["dbg_oht"][:], in_=oht[:])
                    feed(1)
                    # goA = [ps01 | ps23], goB = [ps45 | ps67]
                    goA = mixpsum.tile([128, 1024], f32, tag="mix",
                                       name="goA")
                    goB = mixpsum.tile([128, 1024], f32, tag="mix",
                                       name="goB")
                    nc.tensor.matmul(goB[:, 512:1024], cs["c67b"][:],
                                     oht[:, 1024:1536],
                                     start=True, stop=False)
                    nc.tensor.matmul(goB[:, 512:1024], cs["c67c"][:],
                                     oht[:, 1536:2048],
                                     start=False, stop=False)
                    nc.tensor.matmul(goB[:, 512:1024], cs["c67a"][:],
                                     oht[:, 512:1024],
                                     start=False, stop=True)
                    nc.tensor.matmul(goB[:, 0:512], cs["g45"][:],
                                     oht[0:96, 0:512],
                                     start=True, stop=True)
                    nc.tensor.matmul(goA[:, 0:512], cs["g01"][96:128, :],
                                     oht[96:128, 0:512],
                                     start=True, stop=True,
                                     tile_position=(96, 0))
                    nc.tensor.matmul(goA[:, 512:1024], cs["g23"][96:128, :],
                                     oht[96:128, 0:512],
                                     start=True, stop=True,
                                     tile_position=(96, 0))
                    nc.vector.tensor_copy(stgs[3][:, so:so + 512],
                                          goB[:, 512:1024])
                    nc.scalar.copy(stgs[2][:, so:so + 512], goB[:, 0:512])
                    nc.scalar.copy(stgs[0][:, so:so + 512], goA[:, 0:512])
                    nc.vector.tensor_copy(stgs[1][:, so:so + 512],
                                          goA[:, 512:1024])

                    if s % 4 == 3:
                        for pi in range(4):
                            nc.sync.dma_start(
                                out=out_d[img, pi, :,
                                          half * 2048:(half + 1) * 2048],
                                in_=stgs[pi][:])
                        if s == 3:
                            stgs = []
                            for pi in range(4):
                                stg = outpool.tile([128, 2048], bf16,
                                                   tag=f"st{pi}",
                                                   name=f"st{pi}")
                                stgs.append(stg)

            emit_score_dma(0)
            for tt in range(16):
                emit_score_group(0, tt)
            pend = []

            def feed(k):
                for _ in range(min(k, len(pend))):
                    nim, ntt = pend.pop(0)
                    emit_score_group(nim, ntt)

            for img in range(IMGS):
                if img + 1 < IMGS:
                    emit_score_dma(img + 1)
                    pend = [(img + 1, tt) for tt in range(16)]
                else:
                    pend = []
                idxq = emit_argmin(img)
                feed(8)
                emit_spans(img, idxq, feed)
                feed(16)
    nc.compile()
    _CACHE[key] = nc
    return nc


def _prepare_in_maps(input_data, codebook):
    x = np.ascontiguousarray(np.asarray(input_data, dtype=np.float32))
    assert x.shape == (B, C, H, W)
    consts = _host_consts(np.asarray(codebook, dtype=np.float32))
    xr = x.reshape(B, 64, HWTOK)
    in_maps = []
    for core in range(NCORES):
        sl = slice(core * IMGS, (core + 1) * IMGS)
        m = {"x": np.ascontiguousarray(xr[sl])}
        m.update(consts)
        in_maps.append(m)
    return in_maps


def kernel(input_data, codebook, previous_active_vectors=None,
           num_active_vectors=256, **_):
    from concourse.bass_utils import run_bass_kernel_spmd

    nc = _build()
    in_maps = _prepare_in_maps(input_data, codebook)
    res = run_bass_kernel_spmd(nc, in_maps, core_ids=list(range(NCORES)))
    # per core: [IMGS, 4 lvl-pairs, 128 (=2 lvl x 64 ch), HWTOK] bf16
    outs = [np.asarray(res.results[i]["out"]) for i in range(NCORES)]
    full = np.stack(outs, axis=0).astype(np.float32)
    full = full.reshape(NCORES, IMGS, 4, 2, 64, HWTOK)
    full = full.transpose(2, 3, 0, 1, 4, 5)   # [pair, half, core, img, ch, t]
    return np.ascontiguousarray(full.reshape(NLVL, B, C, H, W))
